# revision 37
# baseline (speedup 1.0000x reference)
"""Trainium2 Bass kernel for the fused sparse-attention block.

Computes (8-core SPMD, head-parallel + final row-shard re-layout):
    qkv = x @ W_qkv; q,k = rope(rmsnorm(q|k)); causal attention;
    out = (attn_out * sigmoid(x @ W_gate + b_gate)) @ W_out

Per core c (heads 2c, 2c+1 for both batches):
  Phase 1: x arrives bf16; XBAR DMA-transpose chunks into xth (no PE
           transposes); fused qkv+gate projection (bf16 matmuls,
           feature-major output); RMSNorm via gpsimd partition_all_reduce
           (no PE reduce/broadcast matmuls) + RoPE on DVE/gpsimd; all of
           qT/kT/gT/v parked in SBUF as bf16 (v re-laid out to natural by
           XBAR DMA transpose).  Gate sigmoid deferred to one sweep at
           phase end so each phase needs a single ACT table.
  Phase 2: per (b,h): scoresT = kT.T-free QK bf16 matmuls into 2-wide
           PSUM tiles, one Exp per 1024 cols, causal mask as a 0/1
           multiply on gpsimd (SBUF-only), PV + ones-denominator PSUM
           accumulation, normalize via reciprocal_approx_fast +
           partition_broadcast, gate multiply, AllToAll bounce buffer.
  Phase 3: one AllToAll per head (head-shard -> row-shard), row-sharded
           output projection with full W_out -> natural [512, 2048].
"""
import sys
if '/opt/trn_rl_repo' not in sys.path:
    sys.path.insert(0, '/opt/trn_rl_repo')

import numpy as np


def _install_ntff_hook_shim():
    """Provide antenv.axon_hooks if the image lacks it (needed only when a
    caller requests NTFF tracing through run_bass_kernel_spmd)."""
    import types
    if 'antenv.axon_hooks' in sys.modules:
        return
    try:
        import antenv
    except ImportError:
        return
    if hasattr(antenv, 'axon_hooks'):
        return
    mod = types.ModuleType('antenv.axon_hooks')
    _state = {}

    def set_axon_ntff_profile_hook(h):
        _state['hook'] = h

    def get_axon_ntff_profile_hook():
        if 'hook' not in _state:
            try:
                from trn_agent_boot.trn_boot import _ntff_profile_via_ctypes
                _state['hook'] = _ntff_profile_via_ctypes('/opt/axon/libaxon_pjrt.so')
            except Exception:
                _state['hook'] = None
        return _state['hook']

    mod.set_axon_ntff_profile_hook = set_axon_ntff_profile_hook
    mod.get_axon_ntff_profile_hook = get_axon_ntff_profile_hook
    sys.modules['antenv.axon_hooks'] = mod
    antenv.axon_hooks = mod


_install_ntff_hook_shim()

B, T, D = 2, 2048, 2048
H = 16
d = 128
N_CORES = 8
HPC = H // N_CORES          # heads per core = 2
ROWS = B * T                # 4096
RC = 512                    # rows per phase-1 chunk
NRC = ROWS // RC            # 8 row chunks
KC = D // 128               # 16 contraction chunks
QKV_CT = 6                  # coltiles: q0 q1 k0 k1 v0 v1
GATE_CT = 2                 # g0 g1
NCT = QKV_CT + GATE_CT      # 8
QCH = 512                   # attention q chunk
EPS = 1e-6
ROPE_BASE = 10000.0
SCALE = 1.0 / np.sqrt(d)

_cache = {}


def _build():
    import concourse.bacc as bacc
    import concourse.mybir as mybir
    from concourse.tile import TileContext

    f32 = mybir.dt.float32
    f32r = mybir.dt.float32r
    bf16 = mybir.dt.bfloat16
    AF = mybir.ActivationFunctionType

    def r_(ap):
        return ap.bitcast(f32r)

    nc = bacc.Bacc("TRN2", target_bir_lowering=False, debug=False,
                   num_devices=N_CORES)

    x_in = nc.dram_tensor("x", [ROWS, D], bf16, kind="ExternalInput").ap()
    w_in = nc.dram_tensor("w_qkvg", [D, NCT * 128], bf16, kind="ExternalInput").ap()
    wout_in = nc.dram_tensor("w_out", [D, D], bf16, kind="ExternalInput").ap()
    bg_in = nc.dram_tensor("b_gate", [128, HPC], f32, kind="ExternalInput").ap()
    cos_in = nc.dram_tensor("costab", [128, T], f32, kind="ExternalInput").ap()
    sin_in = nc.dram_tensor("sintab", [128, T], f32, kind="ExternalInput").ap()
    mask_in = nc.dram_tensor("m01", [128, 4 * QCH], bf16, kind="ExternalInput").ap()
    out_ext = nc.dram_tensor("out", [RC, D], bf16, kind="ExternalOutput").ap()

    with TileContext(nc) as tc:
        with tc.tile_pool(name="persist", bufs=1) as persist, \
             tc.tile_pool(name="dram", bufs=1, space="DRAM") as dram:
            a2a_in = [dram.tile([N_CORES * 128, RC], bf16, name=f"a2a_in{h}")
                      for h in range(HPC)]
            a2a_out = [dram.tile([N_CORES * 128, RC], bf16, name=f"a2a_out{h}")
                       for h in range(HPC)]

            ones_sq = persist.tile([128, 128], bf16, tag="ones_sq")
            ones_sqf = persist.tile([128, 128], f32, tag="ones_sqf")
            eps_col = persist.tile([128, 1], f32, tag="eps")
            bg_sb = persist.tile([128, HPC], f32, tag="bg")
            mask_sb = persist.tile([128, 4 * QCH], bf16, tag="mask")
            cc_sb = persist.tile([128, T], f32, tag="cc")
            ss_sb = persist.tile([128, T], f32, tag="ss")
            # SBUF parks (feature-major, per (head, batch)); v in natural
            # layout [key-tile-major] per head
            qT = [[persist.tile([128, T], bf16, tag=f"qT{h}{b}", name=f"qT{h}{b}")
                   for b in range(B)] for h in range(HPC)]
            kT = [[persist.tile([128, T], bf16, tag=f"kT{h}{b}", name=f"kT{h}{b}")
                   for b in range(B)] for h in range(HPC)]
            gT = [[persist.tile([128, T], bf16, tag=f"gT{h}{b}", name=f"gT{h}{b}")
                   for b in range(B)] for h in range(HPC)]
            v_sb = [persist.tile([128, ROWS], bf16, tag=f"v{h}", name=f"v_sb{h}")
                    for h in range(HPC)]

            nc.sync.dma_start(out=bg_sb[:], in_=bg_in[:])
            nc.sync.dma_start(out=mask_sb[:], in_=mask_in[:])
            nc.sync.dma_start(out=cc_sb[:], in_=cos_in[:])
            nc.sync.dma_start(out=ss_sb[:], in_=sin_in[:])
            nc.vector.memset(ones_sq[:], 1.0)
            nc.vector.memset(ones_sqf[:], 1.0)
            nc.vector.memset(eps_col[:], EPS)

            # ---------------- Phase 1 ----------------
            with tc.tile_pool(name="wq", bufs=1) as wq, \
                 tc.tile_pool(name="p1", bufs=2) as p1, \
                 tc.tile_pool(name="p1xt", bufs=2) as p1xt, \
                 tc.tile_pool(name="pp_pj", bufs=6, space="PSUM") as pp_pj, \
                 tc.tile_pool(name="pp_st", bufs=2, space="PSUM") as pp_st:
                # chunk-0 x transpose first so the PE can start ASAP, then
                # weights: col c of tile (k, ct) at w_sb[:, k*1024 + ct*128 + c]
                xt0 = p1xt.tile([128, KC, RC], bf16, tag="xt", name="xt_0")
                nc.sync.dma_start(out=xt0[:], in_=x_in[0:RC, :], transpose=True)
                w_sb = wq.tile([128, KC * NCT * 128], bf16, tag="w")
                for k in range(KC):
                    nc.sync.dma_start(out=w_sb[:, k * 1024:(k + 1) * 1024],
                                      in_=w_in[k * 128:(k + 1) * 128, :])

                for rc in range(NRC):
                    b = rc // 4
                    t0 = (rc % 4) * RC

                    # fused XBAR DMA transpose: xt[p, k, r] = x[rc*512+r, k*128+p]
                    # Input must be contiguous full rows (strided column
                    # slices corrupt the xbar walk) and ALL xbar transposes
                    # must issue on one queue: each fans out over all 16 DMA
                    # engines, so two in flight interleave and corrupt.
                    if rc == 0:
                        xt = xt0
                    else:
                        xt = p1xt.tile([128, KC, RC], bf16, tag="xt",
                                       name=f"xt{rc}")
                        nc.sync.dma_start(
                            out=xt[:],
                            in_=x_in[rc * RC:(rc + 1) * RC, :],
                            transpose=True)

                    for ct in range(NCT):
                        h = ct % 2
                        ps = pp_pj.tile([128, RC], f32, tag="pj",
                                        name=f"pj{rc}_{ct}")
                        for k in range(KC):
                            nc.tensor.matmul(
                                ps[:],
                                w_sb[:, k * 1024 + ct * 128: k * 1024 + (ct + 1) * 128],
                                xt[:, k, :],
                                start=(k == 0), stop=(k == KC - 1))
                        if ct < 4:
                            # q or k head: rmsnorm (PE reduce+broadcast) + rope
                            isq = ct < 2
                            # rmsnorm: all-ones stationary makes the partition
                            # reduce come out pre-broadcast across partitions
                            sq = p1.tile([128, RC], f32r, tag="sq")
                            nc.scalar.activation(sq[:], ps[:], AF.Square)
                            ssq = pp_st.tile([128, RC], f32, tag="ssq")
                            nc.tensor.matmul(ssq[:], r_(ones_sqf[:]), sq[:],
                                             start=True, stop=True)
                            bc_sb = p1.tile([128, RC], f32, tag="bc_sb")
                            nc.scalar.activation(bc_sb[:], ssq[:],
                                                 AF.Abs_reciprocal_sqrt,
                                                 scale=1.0 / 128.0,
                                                 bias=eps_col[:])
                            qn = p1.tile([128, RC], f32, tag="qn")
                            nc.vector.tensor_mul(qn[:], ps[:], bc_sb[:])
                            # rope: fin = qn*cc + swap(qn)*ss; the partition
                            # swap must be a copy (tensor_tensor ops require
                            # equal start partitions).  (cc = [cos;cos],
                            # ss = [-sin;sin] host tables)
                            sw = p1.tile([128, RC], f32, tag="sw")
                            nc.gpsimd.tensor_copy(sw[0:64, :], qn[64:128, :])
                            nc.gpsimd.tensor_copy(sw[64:128, :], qn[0:64, :])
                            nc.vector.tensor_mul(sw[:], sw[:], ss_sb[:, t0:t0 + RC])
                            nc.vector.tensor_mul(qn[:], qn[:], cc_sb[:, t0:t0 + RC])
                            park = qT if isq else kT
                            nc.vector.tensor_add(
                                park[h][b][:, t0:t0 + RC], qn[:], sw[:])
                        elif ct < 6:
                            # v: evict bf16, re-layout to natural via XBAR DMA
                            sv = p1.tile([128, RC], bf16, tag="sv")
                            nc.scalar.activation(sv[:], ps[:], AF.Copy)
                            vview = v_sb[h][:].rearrange(
                                "p (rt dd) -> p rt dd", dd=128)
                            nc.sync.dma_start(
                                out=vview[:, rc * 4:(rc + 1) * 4, :],
                                in_=sv[:], transpose=True)
                        else:
                            # gate: park raw preactivation; sigmoid swept later
                            nc.scalar.activation(
                                gT[h][b][:, t0:t0 + RC], ps[:], AF.Copy)

            # gate sigmoid sweep (one ACT table switch instead of per-chunk)
            for h in range(HPC):
                for b in range(B):
                    nc.scalar.activation(gT[h][b][:], gT[h][b][:], AF.Sigmoid,
                                         bias=bg_sb[:, h:h + 1])

            # ---------------- Phase 2 + W_out prefetch ----------------
            with tc.tile_pool(name="wout", bufs=4) as woutp:
                wout_tiles = {}

                with tc.tile_pool(name="p2", bufs=2) as p2, \
                     tc.tile_pool(name="p2e", bufs=24) as p2e, \
                     tc.tile_pool(name="pp_s", bufs=4, space="PSUM") as pp_s, \
                     tc.tile_pool(name="pp_o", bufs=2, space="PSUM") as pp_o, \
                     tc.tile_pool(name="pp_d", bufs=2, space="PSUM") as pp_d:
                    for oc in range(2):
                        wt = woutp.tile([128, KC * 512], bf16, tag="wo",
                                        name=f"wo{oc}")
                        for k in range(KC):
                            nc.sync.dma_start(
                                out=wt[:, k * 512:(k + 1) * 512],
                                in_=wout_in[k * 128:(k + 1) * 128,
                                            oc * 512:(oc + 1) * 512])
                        wout_tiles[oc] = wt

                    # pair order (h0,b0),(h1,b0),(h0,b1),(h1,b1): A2A#1 (h0)
                    # fires after the 3rd pair (~75% of phase 2) so pass A
                    # bridges the PE straight into pass B with no idle (an
                    # idle PE also drops out of its fast p-state)
                    for h, b in ((0, 0), (1, 0), (0, 1), (1, 1)):
                        if True:
                            kT_bh = kT[h][b]
                            for qc in range(T // QCH):
                                col0 = qc * QCH
                                qmv = qT[h][b][:, col0:col0 + QCH]
                                o_ps = pp_o.tile([128, QCH], f32, tag="o")
                                den = pp_d.tile([128, QCH], f32, tag="den")
                                nkt = 4 * qc + 4
                                LOOK = 3      # kt-tiles of score/exp lookahead
                                exs = {}
                                # columns < j0(kt) of a diagonal tile are fully
                                # masked: trim score/exp/mask/PV/den to [j0:].
                                def j0_of(kt):
                                    m = kt - 4 * qc
                                    return 128 * m if m > 0 else 0

                                for i in range(nkt + LOOK):
                                    if i < nkt:
                                        kt = i
                                        j0 = j0_of(kt)
                                        sc = pp_s.tile([128, QCH], f32, tag="sc")
                                        nc.tensor.matmul(
                                            sc[:, j0:],
                                            kT_bh[:, kt * 128:(kt + 1) * 128],
                                            qmv[:, j0:], start=True, stop=True)
                                        ex = p2e.tile([128, QCH], bf16, tag="ex")
                                        nc.scalar.activation(ex[:, j0:], sc[:, j0:],
                                                             AF.Exp, scale=SCALE)
                                        m = kt - 4 * qc
                                        if m >= 0:
                                            # causal 0/1 mask on DVE (gpsimd has
                                            # multi-us dispatch latency)
                                            nc.vector.tensor_mul(
                                                ex[:, j0:], ex[:, j0:],
                                                mask_sb[:, m * QCH + j0:
                                                        (m + 1) * QCH])
                                        exs[kt] = ex
                                    if i >= LOOK:
                                        # pv/den interleaved: back-to-back
                                        # accumulates into one PSUM tile stall
                                        # ~565ns, so alternate the two chains
                                        kt = i - LOOK
                                        j0 = j0_of(kt)
                                        ex_p = exs.pop(kt)
                                        nc.tensor.matmul(
                                            o_ps[:, j0:],
                                            v_sb[h][:, (b * 16 + kt) * 128:
                                                    (b * 16 + kt + 1) * 128],
                                            ex_p[:, j0:],
                                            start=(kt == 0), stop=(kt == nkt - 1),
                                            skip_group_check=(j0 > 0))
                                        nc.tensor.matmul(
                                            den[:, j0:], ones_sq[:],
                                            ex_p[:, j0:],
                                            start=(kt == 0), stop=(kt == nkt - 1),
                                            skip_group_check=(j0 > 0))
                                # normalize + gate (den is already broadcast
                                # across partitions by the all-ones stationary)
                                rec = p2.tile([128, QCH], f32, tag="rec")
                                nc.vector.reciprocal_approx_fast(rec[:], den[:])
                                nm = p2.tile([128, QCH], f32, tag="nm")
                                nc.vector.tensor_mul(nm[:], o_ps[:], rec[:])
                                on_sb = p2.tile([128, QCH], bf16, tag="onsb")
                                nc.vector.tensor_mul(
                                    on_sb[:], nm[:], gT[h][b][:, col0:col0 + QCH])
                                shard = b * 4 + qc
                                nc.sync.dma_start(
                                    out=a2a_in[h][shard * 128:(shard + 1) * 128, :],
                                    in_=on_sb[:])
                        if (h, b) in ((0, 1), (1, 1)):
                            nc.gpsimd.collective_compute(
                                "AllToAll", mybir.AluOpType.bypass,
                                replica_groups=[list(range(N_CORES))],
                                ins=[a2a_in[h].opt()], outs=[a2a_out[h].opt()])

                for oc in range(2, 4):
                    wt = woutp.tile([128, KC * 512], bf16, tag="wo", name=f"wo{oc}")
                    for k in range(KC):
                        nc.sync.dma_start(
                            out=wt[:, k * 512:(k + 1) * 512],
                            in_=wout_in[k * 128:(k + 1) * 128, oc * 512:(oc + 1) * 512])
                    wout_tiles[oc] = wt

                # ---------------- Phase 3: output projection ----------------
                with tc.tile_pool(name="p3", bufs=1) as p3, \
                     tc.tile_pool(name="p3e", bufs=3) as p3e, \
                     tc.tile_pool(name="pp_3", bufs=8, space="PSUM") as pp_3:
                    gat = p3.tile([128, KC * RC], bf16, tag="gat")
                    for k in range(0, KC, 2):          # even: head-0 dims (A2A#1)
                        nc.sync.dma_start(
                            out=gat[:, k * RC:(k + 1) * RC],
                            in_=a2a_out[0][(k // 2) * 128:(k // 2 + 1) * 128, :])
                    for k in range(1, KC, 2):          # odd: head-1 dims (A2A#2)
                        nc.sync.dma_start(
                            out=gat[:, k * RC:(k + 1) * RC],
                            in_=a2a_out[1][(k // 2) * 128:(k // 2 + 1) * 128, :])
                    # pass A: even k (head-0 dims, ready after A2A#1) -> SBUF partials
                    partials = {}
                    for oc in range(4):
                        for rt in range(4):
                            ps = pp_3.tile([128, 512], f32, tag="o3",
                                           name=f"psA_{oc}_{rt}")
                            for k in range(0, KC, 2):
                                nc.tensor.matmul(
                                    ps[:], gat[:, k * RC + rt * 128: k * RC + (rt + 1) * 128],
                                    wout_tiles[oc][:, k * 512:(k + 1) * 512],
                                    start=(k == 0), stop=(k == KC - 2))
                            pa = p3.tile([128, 512], bf16, tag=f"pa{oc}{rt}",
                                         name=f"pa_{oc}_{rt}")
                            nc.vector.tensor_copy(pa[:], ps[:])
                            partials[(oc, rt)] = pa
                    # pass B: odd k (head-1 dims, after A2A#2), add partial at evict
                    for oc in range(4):
                        for rt in range(4):
                            ps = pp_3.tile([128, 512], f32, tag="o3",
                                           name=f"psB_{oc}_{rt}")
                            for k in range(1, KC, 2):
                                nc.tensor.matmul(
                                    ps[:], gat[:, k * RC + rt * 128: k * RC + (rt + 1) * 128],
                                    wout_tiles[oc][:, k * 512:(k + 1) * 512],
                                    start=(k == 1), stop=(k == KC - 1))
                            ev = p3e.tile([128, 512], bf16, tag="ev")
                            nc.vector.tensor_add(ev[:], ps[:], partials[(oc, rt)][:])
                            nc.sync.dma_start(
                                out=out_ext[rt * 128:(rt + 1) * 128, oc * 512:(oc + 1) * 512],
                                in_=ev[:])

    nc.compile()
    return nc


def _tables():
    inv = 1.0 / (ROPE_BASE ** (np.arange(0, d, 2, dtype=np.float64) / d))
    pos = np.arange(T, dtype=np.float64)
    ang = pos[None, :] * inv[:, None]          # [64, T]
    cos = np.cos(ang).astype(np.float32)
    sin = np.sin(ang).astype(np.float32)
    cc = np.concatenate([cos, cos], axis=0)    # [128, T]
    ss = np.concatenate([-sin, sin], axis=0)   # [128, T]
    return cc, ss


def kernel(x, W_qkv, W_out, W_gate, b_gate, mask):
    from concourse.bass_utils import run_bass_kernel_spmd
    import ml_dtypes

    if 'nc' not in _cache:
        _cache['nc'] = _build()
    nc = _cache['nc']

    x = np.ascontiguousarray(
        np.asarray(x, dtype=np.float32).reshape(ROWS, D)).astype(ml_dtypes.bfloat16)
    W_qkv = np.asarray(W_qkv, dtype=np.float32)
    W_out = np.ascontiguousarray(np.asarray(W_out, dtype=np.float32)).astype(ml_dtypes.bfloat16)
    W_gate = np.asarray(W_gate, dtype=np.float32)
    b_gate = np.asarray(b_gate, dtype=np.float32)
    cos, sin = _tables()
    f = np.arange(QCH)[None, :]
    p = np.arange(128)[:, None]
    m01 = np.concatenate(
        [np.where(f >= p + 128 * m, 1.0, 0.0) for m in range(4)],
        axis=1).astype(ml_dtypes.bfloat16)

    in_maps = []
    for c in range(N_CORES):
        h0 = HPC * c
        cols = []
        for kind in range(3):                     # q, k, v columns for this core's heads
            for h in range(h0, h0 + HPC):
                cols.append(W_qkv[:, kind * D + h * d:(kind * D + (h + 1) * d)])
        for h in range(h0, h0 + HPC):             # gate columns
            cols.append(W_gate[:, h * d:(h + 1) * d])
        w_qkvg = np.ascontiguousarray(
            np.concatenate(cols, axis=1)).astype(ml_dtypes.bfloat16)
        bg = np.ascontiguousarray(
            b_gate[h0 * d:(h0 + HPC) * d].reshape(HPC, 128).T)
        in_maps.append({
            "x": x, "w_qkvg": w_qkvg, "w_out": W_out, "b_gate": bg,
            "costab": cos, "sintab": sin, "m01": m01,
        })

    res = run_bass_kernel_spmd(nc, in_maps, list(range(N_CORES)))
    _cache['last_results'] = res
    out = np.concatenate(
        [np.asarray(res.results[c]["out"]).astype(np.float32)
         for c in range(N_CORES)], axis=0)
    return out.reshape(B, T, D)
